# revision 49
# baseline (speedup 1.0000x reference)
"""Sparse L1-distance attention (nn_L1AttnSparse) on 8 Trainium2 NeuronCores.

Layout (v3): dst tokens are split across the 8 cores (256 each; 4 blocks of
128 dst, each with 4 slot-quarters of 8 slots).  k/v tables are fp16 with a
host-side feature swap to (w, h) order so every big DVE tensor_tensor keeps
the packed-16-bit 2x mode: the weight / q broadcasts then fall on middle
axes with the 8-wide head dim packed last.  Slot-quarter SWDGE gathers
(non-transpose; 1024 rows x 1KB) land edges on partitions as [dst, slot,
(w h)].  Scores: q-k subtract (DVE 2x), |.| on the Activation engine, then
an in-place pairwise tree over w (DVE 2x) replaces TensorReduce (which has
no fast mode).  Softmax needs no max-subtraction: scores <= 0, a constant
bias keeps exp() in fp16 range and cancels in the normalizer.  The weighted
v-sum is an fp16 multiply (2x) plus slot trees.  A software pipeline keeps
k-gathers two blocks ahead, v-gathers one block ahead, and interleaves the
next block's subtractions into the softmax latency.
"""

import sys

sys.path.insert(0, "/opt/trn_rl_repo")

import numpy as np

import concourse.bass as bass
import concourse.tile as tile
from concourse import bacc, mybir
from concourse.bass_utils import run_bass_kernel_spmd

BS = 2
N_TOK = 2048
NH = 8
W = 64
S = 32  # dst_mxlen
HW = NH * W  # 512 features per (b, tok) row
N_CORES = 8
DT = N_TOK // N_CORES  # dst tokens per core = 256
CHUNKS = DT // 128  # dst chunks of 128 per core = 2
NB = BS * CHUNKS  # blocks per core = 4
SQ = 8  # slots per gather quarter
QTR = SQ * 128  # gathered rows per quarter = 1024
CEXP = 40.0  # constant score bias: exp((CEXP - L)/8), cancels in normalize
SCALE = 1.0 / np.sqrt(W)  # 1/8

# feature swap: table column pos = w*8 + h holds original feature h*64 + w,
# so the head dim (8-wide, packed) is innermost on chip.
_POS = np.arange(HW)
COLPERM = (_POS % NH) * W + _POS // NH  # [pos] -> original feature index


def _wrap_idx(flat):
    """int16 index list -> [128, n/16] tile layout: idx i at [i%16, i//16],
    replicated down the 8 groups of 16 partitions."""
    n = flat.shape[0]
    w16 = np.zeros((16, n // 16), dtype=np.int16)
    w16[np.arange(n) % 16, np.arange(n) // 16] = flat
    return np.tile(w16, (8, 1))


def host_prep(v, q, k, coo, core):
    """Build the per-core input map."""
    srct = np.zeros((N_TOK, S), dtype=np.int64)
    srct[coo[:, 0], coo[:, 2]] = coo[:, 1]
    kf = k.reshape(BS * N_TOK, HW)[:, COLPERM].astype(np.float16)
    vf = v.reshape(BS * N_TOK, HW)[:, COLPERM].astype(np.float16)
    q2 = q.reshape(BS, N_TOK, HW)[:, :, COLPERM].astype(np.float16)

    lo0 = core * DT
    qTh = np.empty((NB, 128, HW), dtype=np.float16)
    idxh = np.empty((NB, 128, 4 * (QTR // 16)), dtype=np.int16)
    for b in range(BS):
        for c in range(CHUNKS):
            blk = b * CHUNKS + c
            lo = lo0 + c * 128
            qTh[blk] = q2[b, lo : lo + 128]
            for qq in range(4):
                sl = slice(qq * SQ, (qq + 1) * SQ)
                flat = (b * N_TOK + srct[lo : lo + 128, sl].T).reshape(-1)
                n16 = QTR // 16
                idxh[blk, :, qq * n16 : (qq + 1) * n16] = _wrap_idx(
                    flat.astype(np.int16)
                )
    return {"kf": kf, "vf": vf, "qT": qTh, "idx": idxh}


def host_unpack(oc, out, core):
    """Scatter one core's oc [NB, 128, HW] into out [BS, N_TOK, HW] fp32."""
    lo0 = core * DT
    for b in range(BS):
        for c in range(CHUNKS):
            blk = b * CHUNKS + c
            lo = lo0 + c * 128
            out[b, lo : lo + 128, COLPERM] = oc[blk].T.astype(np.float32)


def build_kernel():
    nc = bacc.Bacc(
        "TRN2", target_bir_lowering=False, debug=False, num_devices=N_CORES,
        dynamic_dma_scratch_size=32768, num_swdge_queues=1,
    )
    f16 = mybir.dt.float16
    f32 = mybir.dt.float32
    i16 = mybir.dt.int16

    kf = nc.dram_tensor("kf", [BS * N_TOK, HW], f16, kind="ExternalInput").ap()
    vf = nc.dram_tensor("vf", [BS * N_TOK, HW], f16, kind="ExternalInput").ap()
    qT = nc.dram_tensor("qT", [NB, 128, HW], f16, kind="ExternalInput").ap()
    idx = nc.dram_tensor(
        "idx", [NB, 128, 4 * QTR // 16], i16, kind="ExternalInput"
    ).ap()
    oc = nc.dram_tensor("oc", [NB, 128, HW], f16, kind="ExternalOutput").ap()

    with tile.TileContext(nc) as tc:
        with (
            nc.allow_low_precision(reason="fp16 datapath"),
            tc.tile_pool(name="kgp", bufs=2) as kgp,
            tc.tile_pool(name="vgp", bufs=2) as vgp,
            tc.tile_pool(name="small", bufs=2) as smp,
            tc.tile_pool(name="const", bufs=1) as cst,
        ):
            bias_t = cst.tile([128, 1], f32, tag="bias")

            def make_inputs_k(blk):
                st = {}
                it = smp.tile([128, 4 * QTR // 16], i16, tag="idx")
                nc.sync.dma_start(out=it[:], in_=idx[blk])
                qt = smp.tile([128, HW], f16, tag="qt")
                nc.sync.dma_start(out=qt[:], in_=qT[blk])
                kgs = []
                for qq in range(4):
                    kg = kgp.tile([128, SQ, HW], f16, tag=f"kg{qq}")
                    nc.gpsimd.dma_gather(
                        kg[:], kf,
                        it[:, qq * (QTR // 16) : (qq + 1) * (QTR // 16)],
                        QTR, QTR, HW, queue_num=0,
                    )
                    kgs.append(kg)
                st["qt"], st["kgs"], st["idx"] = qt, kgs, it
                return st

            def make_inputs_v(st):
                it = st["idx"]
                vgs = []
                for qq in range(4):
                    vg = vgp.tile([128, SQ, HW], f16, tag=f"vg{qq}")
                    nc.gpsimd.dma_gather(
                        vg[:], vf,
                        it[:, qq * (QTR // 16) : (qq + 1) * (QTR // 16)],
                        QTR, QTR, HW, queue_num=0,
                    )
                    vgs.append(vg)
                st["vgs"] = vgs

            def emit_score(blk, st):
                qt, kgs = st["qt"], st["kgs"]
                E16 = smp.tile([128, S, NH], f16, tag="E")
                st["E16"] = E16
                for qq in range(4):
                    kg = kgs[qq]
                    # kg <- kg - q (broadcast over slots); fp16 2x mode.
                    # q3 of all but the last block runs on the idle GpSimd
                    # engine to offload the DVE bottleneck.
                    sub_eng = (
                        nc.gpsimd if (qq == 3 and blk < NB - 1) else nc.vector
                    )
                    sub_eng.tensor_tensor(
                        out=kg[:], in0=kg[:],
                        in1=qt[:, None, :].to_broadcast([128, SQ, HW]),
                        op=mybir.AluOpType.subtract,
                    )
                    # |diff|: during block 0 the DVE is gather-starved, so
                    # abs runs there via the 4x tensor_scalar path to skip
                    # the ACT round-trip latency; steady-state blocks use ACT
                    if blk == 0:
                        nc.vector.tensor_scalar(
                            out=kg[:], in0=kg[:], scalar1=0.0, scalar2=None,
                            op0=mybir.AluOpType.abs_max,
                        )
                    else:
                        nc.scalar.activation(
                            out=kg[:], in_=kg[:],
                            func=mybir.ActivationFunctionType.Abs,
                        )
                    # L[d, s, h]: in-place pairwise tree over w (fp16 2x)
                    kg4 = kg[:].rearrange("p s (w h) -> p s w h", h=NH)
                    n = W // 2
                    while n >= 1:
                        nc.vector.tensor_tensor(
                            out=kg4[:, :, :n, :], in0=kg4[:, :, :n, :],
                            in1=kg4[:, :, n : 2 * n, :],
                            op=mybir.AluOpType.add,
                        )
                        n //= 2
                    # E = exp((CEXP - L)/8) in fp16
                    nc.scalar.activation(
                        out=E16[:, qq * SQ : (qq + 1) * SQ, :],
                        in_=kg4[:, :, 0, :],
                        func=mybir.ActivationFunctionType.Exp,
                        scale=-SCALE, bias=bias_t[:],
                    )

            def emit_weight(blk, st):
                E16, vgs = st["E16"], st["vgs"]
                # denominator: tree-sum E over slots
                dtr = smp.tile([128, S // 2, NH], f16, tag="dtr")
                nc.vector.tensor_tensor(
                    out=dtr[:], in0=E16[:, : S // 2, :], in1=E16[:, S // 2 :, :],
                    op=mybir.AluOpType.add,
                )
                n = S // 4
                while n >= 2:
                    nc.vector.tensor_tensor(
                        out=dtr[:, :n, :], in0=dtr[:, :n, :],
                        in1=dtr[:, n : 2 * n, :],
                        op=mybir.AluOpType.add,
                    )
                    n //= 2
                den = smp.tile([128, NH], f32, tag="den")
                nc.vector.tensor_tensor(
                    out=den[:], in0=dtr[:, 0, :], in1=dtr[:, 1, :],
                    op=mybir.AluOpType.add,
                )
                rden = smp.tile([128, NH], f16, tag="rden")
                nc.vector.reciprocal(rden[:], den[:])
                # weighted v: vg *= E (broadcast over w), tree-sum slots
                for qq in range(4):
                    vg4 = vgs[qq][:].rearrange("p s (w h) -> p s w h", h=NH)
                    mul_eng = (
                        nc.gpsimd if (qq == 2 and blk < NB - 1) else nc.vector
                    )
                    mul_eng.tensor_tensor(
                        out=vg4, in0=vg4,
                        in1=E16[:, qq * SQ : (qq + 1) * SQ, None, :]
                        .to_broadcast([128, SQ, W, NH]),
                        op=mybir.AluOpType.mult,
                    )
                for qq in range(4):
                    vg4 = vgs[qq][:]
                    n = SQ // 2
                    while n >= 1:
                        nc.vector.tensor_tensor(
                            out=vg4[:, :n], in0=vg4[:, :n],
                            in1=vg4[:, n : 2 * n],
                            op=mybir.AluOpType.add,
                        )
                        n //= 2
                vs01 = smp.tile([128, HW], f16, tag="vs01")
                nc.vector.tensor_tensor(
                    out=vs01[:], in0=vgs[0][:, 0, :], in1=vgs[1][:, 0, :],
                    op=mybir.AluOpType.add,
                )
                vs23 = smp.tile([128, HW], f16, tag="vs23")
                nc.vector.tensor_tensor(
                    out=vs23[:], in0=vgs[2][:, 0, :], in1=vgs[3][:, 0, :],
                    op=mybir.AluOpType.add,
                )
                vsum = smp.tile([128, HW], f16, tag="vsum")
                nc.vector.tensor_tensor(
                    out=vsum[:], in0=vs01[:], in1=vs23[:],
                    op=mybir.AluOpType.add,
                )
                ot = smp.tile([128, HW], f16, tag="ot")
                nc.vector.tensor_tensor(
                    out=ot[:].rearrange("p (w h) -> p w h", h=NH),
                    in0=vsum[:].rearrange("p (w h) -> p w h", h=NH),
                    in1=rden[:, None, :].to_broadcast([128, W, NH]),
                    op=mybir.AluOpType.mult,
                )
                # store on the ACT engine's DGE so SP's in-order queue never
                # delays the next block's idx/q loads behind this store
                nc.scalar.dma_start(out=oc[blk], in_=ot[:])

            # software pipeline: k-gathers two blocks ahead, v one ahead;
            # block N+1's score subs fill block N's softmax latency
            pend = {0: make_inputs_k(0)}
            nc.gpsimd.memset(bias_t[:], CEXP * SCALE)
            pend[1] = make_inputs_k(1)
            make_inputs_v(pend[0])
            emit_score(0, pend[0])
            for blk in range(NB):
                if blk + 1 < NB:
                    make_inputs_v(pend[blk + 1])
                if blk + 2 < NB:
                    pend[blk + 2] = make_inputs_k(blk + 2)
                if blk + 1 < NB:
                    emit_score(blk + 1, pend[blk + 1])
                emit_weight(blk, pend.pop(blk))
    nc.compile()
    return nc


_NC_CACHE = None


def kernel(v, q, k, coo, dst_mxlen):
    global _NC_CACHE
    assert int(dst_mxlen) == S
    v = np.asarray(v, dtype=np.float32)
    q = np.asarray(q, dtype=np.float32)
    k = np.asarray(k, dtype=np.float32)
    coo = np.asarray(coo)

    if _NC_CACHE is None:
        _NC_CACHE = build_kernel()
    nc = _NC_CACHE

    in_maps = [host_prep(v, q, k, coo, core) for core in range(N_CORES)]
    res = run_bass_kernel_spmd(nc, in_maps, list(range(N_CORES)))
    out = np.empty((BS, N_TOK, HW), dtype=np.float32)
    for core in range(N_CORES):
        host_unpack(res.results[core]["oc"], out, core)
    return out.reshape(BS, N_TOK, NH, W)


# revision 50
# speedup vs baseline: 1.0277x; 1.0277x over previous
"""Sparse L1-distance attention (nn_L1AttnSparse) on 8 Trainium2 NeuronCores.

Layout (v3): dst tokens are split across the 8 cores (256 each; 4 blocks of
128 dst, each with 4 slot-quarters of 8 slots).  k/v tables are fp16 with a
host-side feature swap to (w, h) order so every big DVE tensor_tensor keeps
the packed-16-bit 2x mode: the weight / q broadcasts then fall on middle
axes with the 8-wide head dim packed last.  Slot-quarter SWDGE gathers
(non-transpose; 1024 rows x 1KB) land edges on partitions as [dst, slot,
(w h)].  Scores: q-k subtract (DVE 2x), |.| on the Activation engine, then
an in-place pairwise tree over w (DVE 2x) replaces TensorReduce (which has
no fast mode).  Softmax needs no max-subtraction: scores <= 0, a constant
bias keeps exp() in fp16 range and cancels in the normalizer.  The weighted
v-sum is an fp16 multiply (2x) plus slot trees.  A software pipeline keeps
k-gathers two blocks ahead, v-gathers one block ahead, and interleaves the
next block's subtractions into the softmax latency.
"""

import sys

sys.path.insert(0, "/opt/trn_rl_repo")

import numpy as np

import concourse.bass as bass
import concourse.tile as tile
from concourse import bacc, mybir
from concourse.bass_utils import run_bass_kernel_spmd

BS = 2
N_TOK = 2048
NH = 8
W = 64
S = 32  # dst_mxlen
HW = NH * W  # 512 features per (b, tok) row
N_CORES = 8
DT = N_TOK // N_CORES  # dst tokens per core = 256
CHUNKS = DT // 128  # dst chunks of 128 per core = 2
NB = BS * CHUNKS  # blocks per core = 4
SQ = 8  # slots per gather quarter
QTR = SQ * 128  # gathered rows per quarter = 1024
CEXP = 40.0  # constant score bias: exp((CEXP - L)/8), cancels in normalize
SCALE = 1.0 / np.sqrt(W)  # 1/8

# feature swap: table column pos = w*8 + h holds original feature h*64 + w,
# so the head dim (8-wide, packed) is innermost on chip.
_POS = np.arange(HW)
COLPERM = (_POS % NH) * W + _POS // NH  # [pos] -> original feature index


def _wrap_idx(flat):
    """int16 index list -> [128, n/16] tile layout: idx i at [i%16, i//16],
    replicated down the 8 groups of 16 partitions."""
    n = flat.shape[0]
    w16 = np.zeros((16, n // 16), dtype=np.int16)
    w16[np.arange(n) % 16, np.arange(n) // 16] = flat
    return np.tile(w16, (8, 1))


def host_prep(v, q, k, coo, core):
    """Build the per-core input map."""
    srct = np.zeros((N_TOK, S), dtype=np.int64)
    srct[coo[:, 0], coo[:, 2]] = coo[:, 1]
    kf = k.reshape(BS * N_TOK, HW)[:, COLPERM].astype(np.float16)
    vf = v.reshape(BS * N_TOK, HW)[:, COLPERM].astype(np.float16)
    q2 = q.reshape(BS, N_TOK, HW)[:, :, COLPERM].astype(np.float16)

    lo0 = core * DT
    qTh = np.empty((NB, 128, HW), dtype=np.float16)
    idxh = np.empty((NB, 128, 4 * (QTR // 16)), dtype=np.int16)
    for b in range(BS):
        for c in range(CHUNKS):
            blk = b * CHUNKS + c
            lo = lo0 + c * 128
            qTh[blk] = q2[b, lo : lo + 128]
            for qq in range(4):
                sl = slice(qq * SQ, (qq + 1) * SQ)
                flat = (b * N_TOK + srct[lo : lo + 128, sl].T).reshape(-1)
                n16 = QTR // 16
                idxh[blk, :, qq * n16 : (qq + 1) * n16] = _wrap_idx(
                    flat.astype(np.int16)
                )
    return {"kf": kf, "vf": vf, "qT": qTh, "idx": idxh}


def host_unpack(oc, out, core):
    """Scatter one core's oc [NB, 128, HW] into out [BS, N_TOK, HW] fp32."""
    lo0 = core * DT
    for b in range(BS):
        for c in range(CHUNKS):
            blk = b * CHUNKS + c
            lo = lo0 + c * 128
            out[b, lo : lo + 128, COLPERM] = oc[blk].T.astype(np.float32)


def build_kernel():
    nc = bacc.Bacc(
        "TRN2", target_bir_lowering=False, debug=False, num_devices=N_CORES,
        dynamic_dma_scratch_size=32768, num_swdge_queues=1,
    )
    f16 = mybir.dt.float16
    f32 = mybir.dt.float32
    i16 = mybir.dt.int16

    kf = nc.dram_tensor("kf", [BS * N_TOK, HW], f16, kind="ExternalInput").ap()
    vf = nc.dram_tensor("vf", [BS * N_TOK, HW], f16, kind="ExternalInput").ap()
    qT = nc.dram_tensor("qT", [NB, 128, HW], f16, kind="ExternalInput").ap()
    idx = nc.dram_tensor(
        "idx", [NB, 128, 4 * QTR // 16], i16, kind="ExternalInput"
    ).ap()
    oc = nc.dram_tensor("oc", [NB, 128, HW], f16, kind="ExternalOutput").ap()

    with tile.TileContext(nc) as tc:
        with (
            nc.allow_low_precision(reason="fp16 datapath"),
            tc.tile_pool(name="kgp", bufs=2) as kgp,
            tc.tile_pool(name="vgp", bufs=2) as vgp,
            tc.tile_pool(name="small", bufs=2) as smp,
            tc.tile_pool(name="const", bufs=1) as cst,
        ):
            bias_t = cst.tile([128, 1], f32, tag="bias")

            def make_inputs_k(blk):
                st = {}
                it = smp.tile([128, 4 * QTR // 16], i16, tag="idx")
                nc.sync.dma_start(out=it[:], in_=idx[blk])
                qt = smp.tile([128, HW], f16, tag="qt")
                nc.sync.dma_start(out=qt[:], in_=qT[blk])
                kgs = []
                for qq in range(4):
                    kg = kgp.tile([128, SQ, HW], f16, tag=f"kg{qq}")
                    nc.gpsimd.dma_gather(
                        kg[:], kf,
                        it[:, qq * (QTR // 16) : (qq + 1) * (QTR // 16)],
                        QTR, QTR, HW, queue_num=0,
                    )
                    kgs.append(kg)
                st["qt"], st["kgs"], st["idx"] = qt, kgs, it
                return st

            def make_inputs_v(st):
                it = st["idx"]
                vgs = []
                for qq in range(4):
                    vg = vgp.tile([128, SQ, HW], f16, tag=f"vg{qq}")
                    nc.gpsimd.dma_gather(
                        vg[:], vf,
                        it[:, qq * (QTR // 16) : (qq + 1) * (QTR // 16)],
                        QTR, QTR, HW, queue_num=0,
                    )
                    vgs.append(vg)
                st["vgs"] = vgs

            def emit_score(blk, st):
                qt, kgs = st["qt"], st["kgs"]
                E16 = smp.tile([128, S, NH], f16, tag="E")
                st["E16"] = E16
                for qq in range(4):
                    kg = kgs[qq]
                    # kg <- kg - q (broadcast over slots); fp16 2x mode.
                    # q3 of all but the last block runs on the idle GpSimd
                    # engine to offload the DVE bottleneck.
                    sub_eng = (
                        nc.gpsimd if (qq == 3 and blk < NB - 1) else nc.vector
                    )
                    sub_eng.tensor_tensor(
                        out=kg[:], in0=kg[:],
                        in1=qt[:, None, :].to_broadcast([128, SQ, HW]),
                        op=mybir.AluOpType.subtract,
                    )
                    # |diff| on the Activation engine
                    nc.scalar.activation(
                        out=kg[:], in_=kg[:],
                        func=mybir.ActivationFunctionType.Abs,
                    )
                    # L[d, s, h]: in-place pairwise tree over w (fp16 2x)
                    kg4 = kg[:].rearrange("p s (w h) -> p s w h", h=NH)
                    n = W // 2
                    while n >= 1:
                        nc.vector.tensor_tensor(
                            out=kg4[:, :, :n, :], in0=kg4[:, :, :n, :],
                            in1=kg4[:, :, n : 2 * n, :],
                            op=mybir.AluOpType.add,
                        )
                        n //= 2
                    # E = exp((CEXP - L)/8) in fp16
                    nc.scalar.activation(
                        out=E16[:, qq * SQ : (qq + 1) * SQ, :],
                        in_=kg4[:, :, 0, :],
                        func=mybir.ActivationFunctionType.Exp,
                        scale=-SCALE, bias=bias_t[:],
                    )

            def emit_weight(blk, st):
                E16, vgs = st["E16"], st["vgs"]
                # denominator: tree-sum E over slots
                dtr = smp.tile([128, S // 2, NH], f16, tag="dtr")
                nc.vector.tensor_tensor(
                    out=dtr[:], in0=E16[:, : S // 2, :], in1=E16[:, S // 2 :, :],
                    op=mybir.AluOpType.add,
                )
                n = S // 4
                while n >= 2:
                    nc.vector.tensor_tensor(
                        out=dtr[:, :n, :], in0=dtr[:, :n, :],
                        in1=dtr[:, n : 2 * n, :],
                        op=mybir.AluOpType.add,
                    )
                    n //= 2
                den = smp.tile([128, NH], f32, tag="den")
                nc.vector.tensor_tensor(
                    out=den[:], in0=dtr[:, 0, :], in1=dtr[:, 1, :],
                    op=mybir.AluOpType.add,
                )
                rden = smp.tile([128, NH], f16, tag="rden")
                nc.vector.reciprocal(rden[:], den[:])
                # weighted v: vg *= E (broadcast over w), tree-sum slots
                for qq in range(4):
                    vg4 = vgs[qq][:].rearrange("p s (w h) -> p s w h", h=NH)
                    mul_eng = (
                        nc.gpsimd if (qq == 2 and blk < NB - 1) else nc.vector
                    )
                    mul_eng.tensor_tensor(
                        out=vg4, in0=vg4,
                        in1=E16[:, qq * SQ : (qq + 1) * SQ, None, :]
                        .to_broadcast([128, SQ, W, NH]),
                        op=mybir.AluOpType.mult,
                    )
                for qq in range(4):
                    vg4 = vgs[qq][:]
                    n = SQ // 2
                    while n >= 1:
                        nc.vector.tensor_tensor(
                            out=vg4[:, :n], in0=vg4[:, :n],
                            in1=vg4[:, n : 2 * n],
                            op=mybir.AluOpType.add,
                        )
                        n //= 2
                vs01 = smp.tile([128, HW], f16, tag="vs01")
                nc.vector.tensor_tensor(
                    out=vs01[:], in0=vgs[0][:, 0, :], in1=vgs[1][:, 0, :],
                    op=mybir.AluOpType.add,
                )
                vs23 = smp.tile([128, HW], f16, tag="vs23")
                nc.vector.tensor_tensor(
                    out=vs23[:], in0=vgs[2][:, 0, :], in1=vgs[3][:, 0, :],
                    op=mybir.AluOpType.add,
                )
                vsum = smp.tile([128, HW], f16, tag="vsum")
                nc.vector.tensor_tensor(
                    out=vsum[:], in0=vs01[:], in1=vs23[:],
                    op=mybir.AluOpType.add,
                )
                ot = smp.tile([128, HW], f16, tag="ot")
                nc.vector.tensor_tensor(
                    out=ot[:].rearrange("p (w h) -> p w h", h=NH),
                    in0=vsum[:].rearrange("p (w h) -> p w h", h=NH),
                    in1=rden[:, None, :].to_broadcast([128, W, NH]),
                    op=mybir.AluOpType.mult,
                )
                # store on the ACT engine's DGE so SP's in-order queue never
                # delays the next block's idx/q loads behind this store
                nc.scalar.dma_start(out=oc[blk], in_=ot[:])

            # software pipeline: k-gathers two blocks ahead, v one ahead;
            # block N+1's score subs fill block N's softmax latency
            pend = {0: make_inputs_k(0)}
            nc.gpsimd.memset(bias_t[:], CEXP * SCALE)
            pend[1] = make_inputs_k(1)
            make_inputs_v(pend[0])
            emit_score(0, pend[0])
            for blk in range(NB):
                if blk + 1 < NB:
                    make_inputs_v(pend[blk + 1])
                if blk + 2 < NB:
                    pend[blk + 2] = make_inputs_k(blk + 2)
                if blk + 1 < NB:
                    emit_score(blk + 1, pend[blk + 1])
                emit_weight(blk, pend.pop(blk))
    nc.compile()
    return nc


_NC_CACHE = None


def kernel(v, q, k, coo, dst_mxlen):
    global _NC_CACHE
    assert int(dst_mxlen) == S
    v = np.asarray(v, dtype=np.float32)
    q = np.asarray(q, dtype=np.float32)
    k = np.asarray(k, dtype=np.float32)
    coo = np.asarray(coo)

    if _NC_CACHE is None:
        _NC_CACHE = build_kernel()
    nc = _NC_CACHE

    in_maps = [host_prep(v, q, k, coo, core) for core in range(N_CORES)]
    res = run_bass_kernel_spmd(nc, in_maps, list(range(N_CORES)))
    out = np.empty((BS, N_TOK, HW), dtype=np.float32)
    for core in range(N_CORES):
        host_unpack(res.results[core]["oc"], out, core)
    return out.reshape(BS, N_TOK, NH, W)
